# revision 6
# baseline (speedup 1.0000x reference)
"""BotRGCN (2-layer relational GCN) Trainium2 kernel, 8-way SPMD.

Strategy (per sharding hint): nodes sharded contiguously across 8 cores;
edges partitioned by destination core; relation weights replicated.

v2: the per-edge source-row gather runs from an SBUF-resident copy of h
(the HBM random-256B-row gather was the v1 bottleneck: SDMA serialized on
HBM read latency at ~42 GB/s effective).  Per chunk of 8 slots we issue one
SBUF-source transpose-mode SWDGE dma_gather (G[f, i] = h[idx_i, f]), one
HWDGE xbar DMA transpose turning G into 8 edge-major E blocks, then 8
one-hot scatter matmuls into PSUM:  S^T[f, rel*128+dst] += E^T @ A.
Self-loops (root term) are no longer routed through the edge stream: the
root contribution is one dense matmul per tile using an xbar-transposed
copy of the local shard.  A matrices are built on the (otherwise idle) DVE
from host-packed keys/norms.  h and h1 are AllGathered between layers and
re-loaded into the SBUF table.

Self-contained: only imports the system concourse toolchain.
"""
import os
import sys

for _p in ("/opt/trn_rl_repo", "/root/.axon_site/_ro/trn_rl_repo"):
    if os.path.isdir(_p) and _p not in sys.path:
        sys.path.insert(0, _p)

import numpy as np
import ml_dtypes

from concourse import bass, bacc, tile, mybir
from concourse.bass_utils import run_bass_kernel_spmd

BF16 = ml_dtypes.bfloat16

# ---------------- problem constants (hardcoded per spec) ----------------
N_NODES = 50000
N_REL = 3
FEAT = 128
VAL = 16
TEXT = 768
CLASSES = 2
CORES = 8
P = 128           # partition / tile size
W = 64            # one-hot window width
CHMAX = 6         # slots per gather chunk (768 idxs; 1024 overflows the
                  # SWDGE ring in SBUF-source transpose mode)
ABATCH = 16       # slots per A-matrix build batch
KMAX = N_REL * P  # per-tile key space: key = rel*128 + dst_local


# ============================ host planner =============================

def _build_schedule(cts, cmax):
    """Joint (cross-core) slot schedule for one (tile, section).

    cts: list of 8 sorted int arrays (edge keys in [0, cmax)).
    Returns (bases, ranges) where bases[j] is the shared window base of
    slot j and ranges[c][j] = (start, end) into core c's sorted arrays.
    """
    n = len(cts)
    ptrs = [0] * n
    lens = [len(a) for a in cts]
    bases = []
    ranges = [[] for _ in range(n)]
    while any(ptrs[c] < lens[c] for c in range(n)):
        b = min(cts[c][ptrs[c]] for c in range(n) if ptrs[c] < lens[c])
        b = min(int(b), cmax - W)
        bases.append(b)
        for c in range(n):
            s = ptrs[c]
            hi = int(np.searchsorted(cts[c], b + W, side="left"))
            e = min(s + P, hi)
            e = max(e, s)
            ranges[c].append((s, e))
            ptrs[c] = e
    return bases, ranges


class Plan:
    pass


def make_plan(edge_index, edge_type, n_nodes=N_NODES, cores=CORES, lolim=None):
    """Edge partition + joint slot schedule shared by both RGCN layers."""
    pl = Plan()
    pl.cores = cores
    NS = n_nodes // cores
    assert NS * cores == n_nodes
    NSP = ((NS + P - 1) // P) * P
    NT = NSP // P
    NROWS = cores * NSP
    if lolim is None:
        lolim = min(NROWS, 32768)
    hibase = max(0, NROWS - 32768)
    assert hibase <= lolim  # coverage of both windows
    assert hibase % P == 0
    pl.NS, pl.NSP, pl.NT, pl.NROWS = NS, NSP, NT, NROWS
    pl.LOLIM, pl.HIBASE = lolim, hibase

    src = np.asarray(edge_index[0], np.int64)
    dst = np.asarray(edge_index[1], np.int64)
    et = np.asarray(edge_type, np.int64)

    deg = np.zeros((N_REL, n_nodes), np.int64)
    np.add.at(deg, (et, dst), 1)
    anorm = 1.0 / np.maximum(deg[et, dst], 1).astype(np.float32)

    row = (src // NS) * NSP + (src % NS)
    owner = dst // NS
    loc = dst % NS
    tile_id = loc // P
    ct = et * P + (loc % P)           # key = rel*128 + dst_local
    sec = (row >= lolim).astype(np.int64)

    order = np.lexsort((ct, sec, tile_id, owner))
    row, ct, sec, anorm = row[order], ct[order], sec[order], anorm[order]
    owner, tile_id = owner[order], tile_id[order]

    # index boundaries for (core, tile, sec) groups
    key = (owner * NT + tile_id) * 2 + sec
    bounds = np.searchsorted(key, np.arange(cores * NT * 2 + 1))

    def group(c, t, s):
        k = (c * NT + t) * 2 + s
        return bounds[k], bounds[k + 1]

    # per (tile, sec): joint schedule; accumulate per-core slot data
    slot_tile = {0: [], 1: []}      # per section stream: tile of each slot
    slot_base = {0: [], 1: []}
    idx16 = {0: [[] for _ in range(cores)], 1: [[] for _ in range(cores)]}
    keyd = {0: [[] for _ in range(cores)], 1: [[] for _ in range(cores)]}
    nrmd = {0: [[] for _ in range(cores)], 1: [[] for _ in range(cores)]}
    tile_slot_range = {0: np.zeros((NT, 2), np.int64), 1: np.zeros((NT, 2), np.int64)}

    for t in range(NT):
        for s in (0, 1):
            cts, rows_, nrms_ = [], [], []
            for c in range(cores):
                a, b = group(c, t, s)
                cts.append(ct[a:b])
                rows_.append(row[a:b])
                nrms_.append(anorm[a:b])
            start = len(slot_base[s])
            bases, ranges = _build_schedule(cts, KMAX)
            for j, bj in enumerate(bases):
                slot_tile[s].append(t)
                slot_base[s].append(bj)
            for c in range(cores):
                for j, (a, b) in enumerate(ranges[c]):
                    n = b - a
                    ii = np.zeros(P, np.int16)
                    kk = np.full(P, -1.0, np.float32)
                    nn = np.zeros(P, np.float32)
                    r = rows_[c][a:b]
                    if s == 1:
                        r = r - hibase
                    ii[:n] = r.astype(np.int16)
                    kk[:n] = (cts[c][a:b] - bases[j]).astype(np.float32)
                    nn[:n] = nrms_[c][a:b]
                    idx16[s][c].append(ii)
                    keyd[s][c].append(kk)
                    nrmd[s][c].append(nn)
            tile_slot_range[s][t] = (start, len(slot_base[s]))

    pl.NLO = len(slot_base[0])
    pl.NHI = len(slot_base[1])
    pl.NSLOT = pl.NLO + pl.NHI
    pl.slot_base = {s: np.array(slot_base[s], np.int64) for s in (0, 1)}
    pl.slot_tile = {s: np.array(slot_tile[s], np.int64) for s in (0, 1)}
    pl.tile_slot_range = tile_slot_range

    # per-core packed arrays
    pl.idx_wrapped = {}
    pl.keys = {}
    pl.norms = {}
    for c in range(cores):
        parts = []
        for s in (0, 1):
            arr = (np.stack(idx16[s][c]) if idx16[s][c]
                   else np.zeros((0, P), np.int16))
            parts.append(arr)
        pl.idx_wrapped[c] = parts  # list of [nslot, 128] int16 per section
        kk = np.concatenate(
            [np.stack(keyd[s][c]) if keyd[s][c] else np.zeros((0, P), np.float32)
             for s in (0, 1)])
        nn = np.concatenate(
            [np.stack(nrmd[s][c]) if nrmd[s][c] else np.zeros((0, P), np.float32)
             for s in (0, 1)])
        pl.keys[c] = np.ascontiguousarray(kk.T.astype(BF16))    # [128, NSLOT]
        pl.norms[c] = np.ascontiguousarray(nn.T.astype(BF16))   # [128, NSLOT]
    return pl


def wrap16(flat):
    """[L] int16 -> [128, L//16] wrapped layout for dma_gather idxs."""
    L = len(flat)
    assert L % 16 == 0
    a = np.asarray(flat, np.int16).reshape(-1, 16).T  # [16, L//16]
    return np.ascontiguousarray(np.tile(a, (8, 1)))


def blob_layout(pl):
    """Ordered (name, nelem, shape) segments of the single bf16 input blob.
    int16 segments are stored bit-cast as bf16. Offsets 128-elem aligned."""
    NSP, NT, NSLOT = pl.NSP, pl.NT, pl.NSLOT
    NLO, NHI = pl.NLO, pl.NHI
    TC = TEXT // P
    segs = [
        ("textT", [NT, P, TC * P]),
        ("valT", [VAL, NSP]),
        ("fc1w", [VAL, FEAT]),
        ("fc2w", [P, TC * P]),
        ("rwv", [FEAT, FEAT]),
        ("rwt", [FEAT, FEAT]),
        ("beff", [1, FEAT]),
        ("ww1", [P, N_REL * FEAT]),
        ("wroot1", [P, FEAT]),
        ("b1", [1, FEAT]),
        ("ww2", [P, N_REL * FEAT]),
        ("wroot2", [P, FEAT]),
        ("b2", [1, FEAT]),
        ("fc3w", [FEAT, CLASSES]),
        ("fc3b", [1, CLASSES]),
        ("iota", [P, W]),
        ("ones1", [1, P]),
        ("keys", [P, max(NSLOT, 1)]),
        ("norms", [P, max(NSLOT, 1)]),
        ("idxlo", [P, max(NLO, 1) * 8]),
        ("idxhi", [P, max(NHI, 1) * 8]),
    ]
    out = {}
    off = 0
    for name, shape in segs:
        n = int(np.prod(shape))
        out[name] = (off, n, shape)
        off += ((n + 127) // 128) * 128
    return out, off

# ============================ bass builder =============================

def build_bass(pl, ablate=()):
    ab = set(ablate)
    NSP, NT = pl.NSP, pl.NT
    NROWS = pl.NROWS
    NLO, NHI, NSLOT = pl.NLO, pl.NHI, pl.NSLOT
    TC = TEXT // P  # text chunks
    NRANK = NROWS // P          # 392 h-table ranks (256B per rank-row)
    HIRANK = pl.HIBASE // P     # rank offset of the hi gather window

    cores = getattr(pl, "cores", CORES)
    nc = bacc.Bacc("TRN2", target_bir_lowering=False, debug=False,
                   num_devices=cores, num_swdge_queues=4)
    qrr = {"n": 0}  # round-robin SWDGE queue picker
    dt = mybir.dt
    f32, bf, i16 = dt.float32, dt.bfloat16, dt.int16

    # ---- parameters: one packed bf16 blob + output
    layout, blob_n = blob_layout(pl)
    p_blob = nc.declare_dram_parameter("blob", [1, blob_n], bf, isOutput=False)
    p_logT = nc.declare_dram_parameter("logitsT", [CLASSES, NSP], f32, isOutput=True)

    def seg(name, dtype=bf):
        off, n, shape = layout[name]
        ap = p_blob[0:1, off:off + n]
        if dtype != bf:
            ap = ap.bitcast(dtype)
        r = int(np.prod(shape[:-1]))
        return ap.rearrange("o (r c) -> (o r) c", r=r)

    with tile.TileContext(nc) as tc:
        with tc.tile_pool(name="wt", bufs=1) as wt, \
             tc.tile_pool(name="sb", bufs=2) as sb, \
             tc.tile_pool(name="elo", bufs=3) as elo, \
             tc.tile_pool(name="ehi", bufs=3) as ehi, \
             tc.tile_pool(name="tts", bufs=3) as tts, \
             tc.tile_pool(name="dram", bufs=1, space="DRAM") as dram:

            # ---- resident weights / tables
            def resident(name, dtype=bf):
                off, n, shape = layout[name]
                t = wt.tile(list(shape[-2:] if len(shape) == 2 else shape), dtype,
                            tag=name)
                nc.sync.dma_start(t[:], seg(name, dtype))
                return t

            fc1w = resident("fc1w")
            fc2w = resident("fc2w")
            rwv = resident("rwv")
            rwt = resident("rwt")
            beff = resident("beff")
            ww1 = resident("ww1")
            wroot1 = resident("wroot1")
            b1 = resident("b1")
            ww2 = resident("ww2")
            wroot2 = resident("wroot2")
            b2 = resident("b2")
            fc3w = resident("fc3w")
            fc3b = resident("fc3b")
            iota = resident("iota")
            ones1 = resident("ones1")
            valT = resident("valT")
            keys = resident("keys")
            norms = resident("norms")
            idxsb = [resident("idxlo", i16), resident("idxhi", i16)]

            # ---- SBUF-resident h table + staging
            htab = wt.tile([P, NRANK * P], bf, tag="htab")   # h[row] @ part row%128, rank row//128
            hstage = wt.tile([P, NT * P], bf, tag="hstage")  # local shard, partition-major
            hT = wt.tile([P, NT, P], bf, tag="hT")           # hT[:, t, :] = local tile t transposed

            # ---- DRAM intermediates (partition-major shard layout [128, NT*128])
            h_shard = dram.tile([P, NT * P], bf)
            _as = "Shared" if (cores > 1 and "coll" not in ab) else "Local"
            h_full = dram.tile([cores * P, NT * P], bf, addr_space=_as)
            h1_shard = dram.tile([P, NT * P], bf)
            h1_full = dram.tile([cores * P, NT * P], bf, addr_space=_as)

            # ================= phase 1: feature MLP =================
            with tc.tile_pool(name="ps1", bufs=2, space="PSUM") as ps1:
                for t in range(NT):
                    tt = tts.tile([P, TC, P], bf, tag="tt")
                    toff = layout["textT"][0] + t * P * TC * P
                    nc.sync.dma_start(
                        tt[:], p_blob[0:1, toff:toff + P * TC * P]
                        .rearrange("o (p c n) -> (o p) c n", p=P, c=TC))
                    pvT = ps1.tile([P, P], f32, tag="pvT", space="PSUM")
                    nc.tensor.matmul(out=pvT[:], lhsT=fc1w[:],
                                     rhs=valT[:, t * P:(t + 1) * P],
                                     start=True, stop=True)
                    vT = sb.tile([P, P], bf, tag="vT")
                    nc.vector.tensor_copy(out=vT[:], in_=pvT[:])
                    ptT = ps1.tile([P, P], f32, tag="ptT", space="PSUM")
                    for c in range(TC):
                        nc.tensor.matmul(out=ptT[:],
                                         lhsT=fc2w[:, c * P:(c + 1) * P],
                                         rhs=tt[:, c, :],
                                         start=(c == 0), stop=(c == TC - 1))
                    tT = sb.tile([P, P], bf, tag="tT")
                    nc.vector.tensor_copy(out=tT[:], in_=ptT[:])
                    ph = ps1.tile([P, P], f32, tag="ph", space="PSUM")
                    nc.tensor.matmul(out=ph[:], lhsT=vT[:], rhs=rwv[:],
                                     start=True, stop=False)
                    nc.tensor.matmul(out=ph[:], lhsT=tT[:], rhs=rwt[:],
                                     start=False, stop=False)
                    nc.tensor.matmul(out=ph[:], lhsT=ones1[:], rhs=beff[:],
                                     start=False, stop=True)
                    lk = sb.tile([P, P], f32, tag="lk")
                    nc.vector.tensor_scalar(out=lk[:], in0=ph[:], scalar1=0.01,
                                            scalar2=None, op0=mybir.AluOpType.mult)
                    nc.vector.tensor_tensor(out=hstage[:, t * P:(t + 1) * P],
                                            in0=ph[:], in1=lk[:],
                                            op=mybir.AluOpType.max)

            def share_h(stage_src, shard_dram, full_dram):
                """stage (SBUF, local shard) -> AllGather -> htab + hT."""
                nc.sync.dma_start(shard_dram[:], stage_src[:])
                if cores > 1 and "coll" not in ab:
                    nc.gpsimd.collective_compute(
                        "AllGather", mybir.AluOpType.bypass,
                        replica_groups=[list(range(cores))],
                        ins=[shard_dram.opt()], outs=[full_dram.opt()])
                else:
                    nc.sync.dma_start(full_dram[0:P, :], shard_dram[:])
                # htab[p, s*NT*128 + x] = full[s*128 + p, x]
                nc.sync.dma_start(
                    htab[:].rearrange("p (s x) -> p s x", s=cores),
                    full_dram[:].rearrange("(s p) x -> p s x", s=cores))
                # local-tile transposes for the root term
                nc.scalar.dma_start(hT[:], stage_src[:].rearrange("p (t f) -> p t f", t=NT),
                                    transpose=True)

            share_h(hstage, h_shard, h_full)

            # ================= RGCN layers =================
            def rgcn_layer(ww, wroot, bb, layer, out_stage):
                emitted = {0: -1, 1: -1}   # last emitted gather chunk per stream
                xemitted = {0: -1, 1: -1}  # last emitted xbar transpose per stream
                aemitted = {0: -1, 1: -1}  # last emitted A batch per stream
                gbufs = {0: {}, 1: {}}     # chunk id -> G tile
                ebufs = {0: {}, 1: {}}     # chunk id -> (E tile, s0, ns)
                abufs = {0: {}, 1: {}}     # batch id -> (A tile, s0, ns)
                pools = {0: elo, 1: ehi}
                nstream = {0: NLO, 1: NHI}

                def emit_chunk(s, ci):
                    s0 = ci * CHMAX
                    ns = min(CHMAX, nstream[s] - s0)
                    gt = pools[s].tile([P, 1, CHMAX * P], bf, tag=f"g{s}")
                    if "gather" in ab:
                        nc.vector.memset(gt[:, 0:1, 0:2], 0.0)
                        gbufs[s][ci] = gt
                        gbufs[s].pop(ci - 3, None)
                        return
                    if s == 0:
                        src_ap = htab[:]
                    else:
                        src_ap = htab[:, HIRANK * P:]
                    qrr["n"] += 1
                    nc.gpsimd.dma_gather(
                        out_ap=gt[:, :, 0:ns * P],
                        in_ap=src_ap,
                        idxs_ap=idxsb[s][:, s0 * 8:(s0 + ns) * 8],
                        num_idxs=ns * P,
                        num_idxs_reg=ns * P,
                        elem_size=FEAT,
                        transpose=True,
                        sbuf_tokens_per_rank=P,
                        sbuf_free_dim_per_rank=2 * P,
                        queue_num=qrr["n"] % 4)
                    gbufs[s][ci] = gt
                    gbufs[s].pop(ci - 3, None)

                def emit_xpose(s, ci):
                    s0 = ci * CHMAX
                    ns = min(CHMAX, nstream[s] - s0)
                    gt = gbufs[s][ci]
                    et = pools[s].tile([P, CHMAX, FEAT], bf, tag=f"e{s}")
                    if "xpose" in ab:
                        nc.vector.memset(et[:, 0:1, 0:2], 0.0)
                    else:
                        nc.sync.dma_start(et[:, 0:ns, :], gt[:, 0, 0:ns * P],
                                          transpose=True)
                    ebufs[s][ci] = (et, s0, ns)
                    ebufs[s].pop(ci - 3, None)

                def emit_abatch(s, ai):
                    s0 = ai * ABATCH
                    ns = min(ABATCH, nstream[s] - s0)
                    at = pools[s].tile([P, ABATCH, W], bf, tag=f"a{s}")
                    if "abuild" in ab:
                        nc.vector.memset(at[:, 0:1, 0:2], 0.0)
                        abufs[s][ai] = (at, s0, ns)
                        abufs[s].pop(ai - 3, None)
                        return
                    g0 = s0 + (0 if s == 0 else NLO)
                    kb = keys[:, g0:g0 + ns].unsqueeze(2).to_broadcast([P, ns, W])
                    nb = norms[:, g0:g0 + ns].unsqueeze(2).to_broadcast([P, ns, W])
                    ib = iota[:].unsqueeze(1).to_broadcast([P, ns, W])
                    nc.vector.tensor_tensor(out=at[:, 0:ns, :], in0=ib, in1=kb,
                                            op=mybir.AluOpType.is_equal)
                    nc.vector.tensor_tensor(out=at[:, 0:ns, :], in0=at[:, 0:ns, :],
                                            in1=nb, op=mybir.AluOpType.mult)
                    abufs[s][ai] = (at, s0, ns)
                    abufs[s].pop(ai - 3, None)

                with tc.tile_pool(name=f"psl{layer}", bufs=2, space="PSUM") as psl:
                    for t in range(NT):
                        # [P, 512] so each buf is exactly one 2KB PSUM bank
                        pS = psl.tile([P, 4 * P], f32, tag="pS", space="PSUM")
                        if "memset" not in ab:
                            nc.vector.memset(pS[:, 0:KMAX], 0.0)
                        for s in (0, 1):
                            a, b = pl.tile_slot_range[s][t]
                            for j in range(a, b):
                                ci = j // CHMAX
                                ai = j // ABATCH
                                if ci > emitted[s]:
                                    emit_chunk(s, ci)
                                    emitted[s] = ci
                                if ci > xemitted[s]:
                                    emit_xpose(s, ci)
                                    xemitted[s] = ci
                                if ai > aemitted[s]:
                                    emit_abatch(s, ai)
                                    aemitted[s] = ai
                                et, es0, _ = ebufs[s][ci]
                                at, as0, _ = abufs[s][ai]
                                bj = int(pl.slot_base[s][j])
                                if "slotmm" in ab:
                                    continue
                                nc.tensor.matmul(
                                    out=pS[:, bj:bj + W],
                                    lhsT=et[:, j - es0, :], rhs=at[:, j - as0, :],
                                    start=False, stop=False,
                                    skip_group_check=True)
                        sS = sb.tile([P, KMAX], bf, tag="sS")
                        nc.scalar.activation(out=sS[:], in_=pS[:, 0:KMAX],
                                             func=mybir.ActivationFunctionType.Copy)
                        if layer == 1:
                            pO = psl.tile([P, FEAT], f32, tag="pO", space="PSUM")
                            for r in range(N_REL):
                                nc.tensor.matmul(out=pO[:],
                                                 lhsT=sS[:, r * P:(r + 1) * P],
                                                 rhs=ww[:, r * FEAT:(r + 1) * FEAT],
                                                 start=(r == 0), stop=False)
                            nc.tensor.matmul(out=pO[:], lhsT=hT[:, t, :],
                                             rhs=wroot[:], start=False, stop=False)
                            nc.tensor.matmul(out=pO[:], lhsT=ones1[:], rhs=bb[:],
                                             start=False, stop=True)
                            nc.vector.tensor_copy(
                                out=out_stage[:, t * P:(t + 1) * P], in_=pO[:])
                        else:
                            pO = psl.tile([P, P], f32, tag="pO", space="PSUM")
                            for r in range(N_REL):
                                nc.tensor.matmul(out=pO[:],
                                                 lhsT=ww[:, r * FEAT:(r + 1) * FEAT],
                                                 rhs=sS[:, r * P:(r + 1) * P],
                                                 start=(r == 0), stop=False)
                            nc.tensor.matmul(out=pO[:], lhsT=wroot[:],
                                             rhs=hT[:, t, :], start=False, stop=False)
                            nc.tensor.matmul(out=pO[:], lhsT=bb[:], rhs=ones1[:],
                                             start=False, stop=True)
                            h2T = sb.tile([P, P], bf, tag="h2T")
                            nc.vector.tensor_copy(out=h2T[:], in_=pO[:])
                            pL = psl.tile([CLASSES, P], f32, tag="pL", space="PSUM")
                            nc.tensor.matmul(out=pL[:], lhsT=fc3w[:], rhs=h2T[:],
                                             start=True, stop=False)
                            nc.tensor.matmul(out=pL[:], lhsT=fc3b[:], rhs=ones1[:],
                                             start=False, stop=True)
                            lg = sb.tile([CLASSES, P], f32, tag="lg")
                            nc.vector.tensor_copy(out=lg[:], in_=pL[:])
                            nc.sync.dma_start(p_logT[:, t * P:(t + 1) * P], lg[:])

            rgcn_layer(ww1, wroot1, b1, 1, hstage)
            share_h(hstage, h1_shard, h1_full)
            rgcn_layer(ww2, wroot2, b2, 2, None)

    nc.compile()
    return nc


# ============================ host packing =============================

def pack_inputs(pl, inputs):
    """Build per-core in_maps from the full problem inputs."""
    NS, NSP, NT = pl.NS, pl.NSP, pl.NT
    TC = TEXT // P

    vf = np.asarray(inputs["value_feature"], np.float32)
    tf = np.asarray(inputs["text_feature"], np.float32)

    def shard_textT(c):
        x = np.zeros((NSP, TEXT), np.float32)
        x[:NS] = tf[c * NS:(c + 1) * NS]
        # [NT, 128p(k within chunk), TC, 128n] -> flat [NT, 128, TC*128]
        y = x.reshape(NT, P, TC, P).transpose(0, 3, 2, 1)
        return np.ascontiguousarray(y.reshape(NT, P, TC * P).astype(BF16))

    def shard_valT(c):
        x = np.zeros((NSP, VAL), np.float32)
        x[:NS] = vf[c * NS:(c + 1) * NS]
        return np.ascontiguousarray(x.T.astype(BF16))

    f32 = np.float32
    fc1w = np.asarray(inputs["fc1_w"], f32)
    fc2w = np.asarray(inputs["fc2_w"], f32)
    relw = np.asarray(inputs["relu_w"], f32)
    beff = (np.concatenate([np.asarray(inputs["fc1_b"], f32),
                            np.asarray(inputs["fc2_b"], f32)]) @ relw
            + np.asarray(inputs["relu_b"], f32))
    # fc2w host layout [128 k, TC*128 f]: [k, c*128+f] = fc2_w[c*128+k, f]
    fc2w_t = np.ascontiguousarray(
        fc2w.reshape(TC, P, FEAT).transpose(1, 0, 2).reshape(P, TC * FEAT).astype(BF16))

    def stack_w(wrel):
        w = np.asarray(wrel, f32)  # [3,128,128]
        return np.ascontiguousarray(
            w.transpose(1, 0, 2).reshape(P, N_REL * FEAT).astype(BF16))

    layout, blob_n = blob_layout(pl)
    shared = dict(
        fc1w=fc1w.astype(BF16), fc2w=fc2w_t,
        rwv=np.ascontiguousarray(relw[:FEAT].astype(BF16)),
        rwt=np.ascontiguousarray(relw[FEAT:].astype(BF16)),
        beff=beff[None].astype(BF16),
        ww1=stack_w(inputs["rgcn1_wrel"]),
        wroot1=np.asarray(inputs["rgcn1_wroot"], f32).astype(BF16),
        b1=np.asarray(inputs["rgcn1_b"], f32)[None].astype(BF16),
        ww2=stack_w(inputs["rgcn2_wrel"]),
        wroot2=np.asarray(inputs["rgcn2_wroot"], f32).astype(BF16),
        b2=np.asarray(inputs["rgcn2_b"], f32)[None].astype(BF16),
        fc3w=np.asarray(inputs["fc3_w"], f32).astype(BF16),
        fc3b=np.asarray(inputs["fc3_b"], f32)[None].astype(BF16),
        iota=np.tile(np.arange(W, dtype=f32), (P, 1)).astype(BF16),
        ones1=np.ones((1, P), f32).astype(BF16),
    )

    in_maps = []
    for c in range(CORES):
        lo, hi = pl.idx_wrapped[c]
        vals = dict(shared)
        vals["textT"] = shard_textT(c)
        vals["valT"] = shard_valT(c)
        vals["idxlo"] = (wrap16(lo.reshape(-1)) if lo.size
                         else np.zeros((P, 8), np.int16)).view(BF16)
        vals["idxhi"] = (wrap16(hi.reshape(-1)) if hi.size
                         else np.zeros((P, 8), np.int16)).view(BF16)
        vals["keys"] = pl.keys[c] if pl.NSLOT else np.zeros((P, 1), BF16)
        vals["norms"] = pl.norms[c] if pl.NSLOT else np.zeros((P, 1), BF16)
        blob = np.zeros((1, blob_n), BF16)
        for name, (off, n, shape) in layout.items():
            a = vals[name]
            assert a.size == n, (name, a.shape, shape)
            blob[0, off:off + n] = a.reshape(-1)
        in_maps.append({"blob": blob})
    return in_maps


# ============================ entry point =============================

_cache = {}


def kernel(**inputs):
    ei = np.asarray(inputs["edge_index"], np.int64)
    et = np.asarray(inputs["edge_type"], np.int64)
    idx = np.asarray(inputs["idx"], np.int64)

    key = hash((ei.tobytes(), et.tobytes()))
    if key not in _cache:
        pl = make_plan(ei, et)
        nc = build_bass(pl)
        _cache[key] = (pl, nc)
    pl, nc = _cache[key]

    in_maps = pack_inputs(pl, inputs)
    res = run_bass_kernel_spmd(nc, in_maps, list(range(CORES)))

    NS, NSP = pl.NS, pl.NSP
    logits = np.zeros((N_NODES, CLASSES), np.float32)
    for c in range(CORES):
        lt = res.results[c]["logitsT"]  # [2, NSP]
        logits[c * NS:(c + 1) * NS] = lt[:, :NS].T
    out = logits[idx]
    return out.astype(np.float32)


# revision 12
# speedup vs baseline: 5.1318x; 5.1318x over previous
"""BotRGCN (2-layer relational GCN) Trainium2 kernel, 8-way SPMD.

Strategy (per sharding hint): nodes sharded contiguously across 8 cores;
edges partitioned by destination core; relation weights replicated.

v3: the binding resource is SWDGE descriptor generation on the GpSimd Q7
(~7ns per gathered row, serialized), so the design minimizes gathered
lanes: (a) self-loops never enter the edge stream -- the root term is one
dense matmul per tile against an xbar-transposed copy of the local shard;
(b) layer 2 aggregates only at the 10000 output nodes (`idx`), remapped to
compact per-core "virtual tiles" whose logits the host scatters back;
(c) destination tiles are scheduled in pairs sharing one PSUM window so
slot fill improves.  Per 128-edge slot, one SWDGE dma_gather pulls source
rows from the AllGathered h table in DRAM (edge-major, no transpose
needed) and one PE matmul with a DVE-built one-hot A scatters them into
PSUM: S^T[f, key] += E^T @ A, key = rel*128 + dst_lane.

Self-contained: only imports the system concourse toolchain.
"""
import os
import sys

for _p in ("/opt/trn_rl_repo", "/root/.axon_site/_ro/trn_rl_repo"):
    if os.path.isdir(_p) and _p not in sys.path:
        sys.path.insert(0, _p)

import numpy as np
import ml_dtypes

from concourse import bass, bacc, tile, mybir
from concourse.bass_utils import run_bass_kernel_spmd

BF16 = ml_dtypes.bfloat16

# ---------------- problem constants (hardcoded per spec) ----------------
N_NODES = 50000
N_REL = 3
FEAT = 128
VAL = 16
TEXT = 768
CLASSES = 2
CORES = 8
P = 128           # partition / tile size
W = 64            # one-hot window width
CHMAX = 8         # slots per gather chunk (1024 idxs = SWDGE ring cap)
ABATCH = 16       # slots per A-matrix build batch
TKEY = N_REL * P  # per-tile key space: key = rel*128 + dst_lane
GRP = 2           # dst tiles per PSUM supergroup
SKEY = 512        # key stride between tiles in a group (PSUM bank aligned so
                  # no W-window ever crosses a 2KB bank boundary)
GKEY = (GRP - 1) * SKEY + TKEY


# ============================ host planner =============================

def _build_schedule(cts, cmax):
    """Joint (cross-core) slot schedule for one (group, section).

    cts: list of 8 sorted int arrays (edge keys in [0, cmax)).
    Returns (bases, ranges) where bases[j] is the shared window base of
    slot j and ranges[c][j] = (start, end) into core c's sorted arrays.
    """
    n = len(cts)
    ptrs = [0] * n
    lens = [len(a) for a in cts]
    bases = []
    ranges = [[] for _ in range(n)]
    while any(ptrs[c] < lens[c] for c in range(n)):
        b = min(cts[c][ptrs[c]] for c in range(n) if ptrs[c] < lens[c])
        b = min(int(b), cmax - W)
        bases.append(b)
        for c in range(n):
            s = ptrs[c]
            hi = int(np.searchsorted(cts[c], b + W, side="left"))
            e = min(s + P, hi)
            e = max(e, s)
            ranges[c].append((s, e))
            ptrs[c] = e
    return bases, ranges


class Layer:
    pass


class Plan:
    pass


def _schedule_layer(row, ct, sec, owner, group, n_groups, cores, key_span,
                    hibase):
    """Joint slot schedule over (group, sec); returns a Layer."""
    L = Layer()
    L.n_groups = n_groups
    order = np.lexsort((ct, sec, group, owner))
    row, ct, sec, owner, group = (a[order] for a in (row, ct, sec, owner, group))
    norm = _schedule_layer.norm[order]

    key = (owner * n_groups + group) * 2 + sec
    bounds = np.searchsorted(key, np.arange(cores * n_groups * 2 + 1))

    def grp(c, g, s):
        k = (c * n_groups + g) * 2 + s
        return bounds[k], bounds[k + 1]

    slot_base = {0: [], 1: []}
    idx16 = {0: [[] for _ in range(cores)], 1: [[] for _ in range(cores)]}
    keyd = {0: [[] for _ in range(cores)], 1: [[] for _ in range(cores)]}
    nrmd = {0: [[] for _ in range(cores)], 1: [[] for _ in range(cores)]}
    group_slot_range = {0: np.zeros((n_groups, 2), np.int64),
                        1: np.zeros((n_groups, 2), np.int64)}

    for g in range(n_groups):
        for s in (0, 1):
            cts, rows_, nrms_ = [], [], []
            for c in range(cores):
                a, b = grp(c, g, s)
                cts.append(ct[a:b])
                rows_.append(row[a:b])
                nrms_.append(norm[a:b])
            start = len(slot_base[s])
            bases, ranges = _build_schedule(cts, key_span)
            for bj in bases:
                slot_base[s].append(bj)
            for c in range(cores):
                for j, (a, b) in enumerate(ranges[c]):
                    n = b - a
                    ii = np.zeros(P, np.int16)
                    kk = np.full(P, -1.0, np.float32)
                    nn = np.zeros(P, np.float32)
                    r = rows_[c][a:b]
                    if s == 1:
                        r = r - hibase
                    ii[:n] = r.astype(np.int16)
                    kk[:n] = (cts[c][a:b] - bases[j]).astype(np.float32)
                    nn[:n] = nrms_[c][a:b]
                    idx16[s][c].append(ii)
                    keyd[s][c].append(kk)
                    nrmd[s][c].append(nn)
            group_slot_range[s][g] = (start, len(slot_base[s]))

    L.NLO = len(slot_base[0])
    L.NHI = len(slot_base[1])
    L.NSLOT = L.NLO + L.NHI
    L.slot_base = {s: np.array(slot_base[s], np.int64) for s in (0, 1)}
    L.group_slot_range = group_slot_range
    L.idx_wrapped = {c: [(np.stack(idx16[s][c]) if idx16[s][c]
                          else np.zeros((0, P), np.int16)) for s in (0, 1)]
                     for c in range(cores)}
    L.keys = {}
    L.norms = {}
    for c in range(cores):
        kk = np.concatenate(
            [np.stack(keyd[s][c]) if keyd[s][c] else np.zeros((0, P), np.float32)
             for s in (0, 1)])
        nn = np.concatenate(
            [np.stack(nrmd[s][c]) if nrmd[s][c] else np.zeros((0, P), np.float32)
             for s in (0, 1)])
        L.keys[c] = np.ascontiguousarray(kk.T.astype(BF16))   # [128, NSLOT]
        L.norms[c] = np.ascontiguousarray(nn.T.astype(BF16))  # [128, NSLOT]
    return L


def make_plan(edge_index, edge_type, out_idx, n_nodes=N_NODES, cores=CORES,
              lolim=None):
    pl = Plan()
    pl.cores = cores
    NS = n_nodes // cores
    assert NS * cores == n_nodes
    NSP = ((NS + P - 1) // P) * P
    NT = NSP // P
    NROWS = cores * NSP
    if lolim is None:
        lolim = min(NROWS, 32768)
    hibase = max(0, NROWS - 32768)
    assert hibase <= lolim and hibase % P == 0
    pl.NS, pl.NSP, pl.NT, pl.NROWS = NS, NSP, NT, NROWS
    pl.LOLIM, pl.HIBASE = lolim, hibase
    pl.NG1 = (NT + GRP - 1) // GRP

    src = np.asarray(edge_index[0], np.int64)
    dst = np.asarray(edge_index[1], np.int64)
    et = np.asarray(edge_type, np.int64)

    deg = np.zeros((N_REL, n_nodes), np.int64)
    np.add.at(deg, (et, dst), 1)
    norm = 1.0 / np.maximum(deg[et, dst], 1).astype(np.float32)

    row = (src // NS) * NSP + (src % NS)
    owner = dst // NS
    loc = dst % NS
    tile_id = loc // P
    sec = (row >= lolim).astype(np.int64)

    # ---- layer 1: all edges, groups = tile pairs
    ct1 = (tile_id % GRP) * SKEY + et * P + (loc % P)
    _schedule_layer.norm = norm
    pl.L1 = _schedule_layer(row, ct1, sec, owner, tile_id // GRP, pl.NG1,
                            cores, GKEY, hibase)

    # ---- layer 2: only edges into `out_idx` nodes, compact virtual tiles
    oi = np.unique(np.asarray(out_idx, np.int64))
    pl.active = {}   # core -> sorted local node ids
    lane = np.full(n_nodes, -1, np.int64)   # node -> global lane on its core
    nact = []
    for c in range(cores):
        a = oi[(oi >= c * NS) & (oi < (c + 1) * NS)] - c * NS
        pl.active[c] = a
        lane[c * NS + a] = np.arange(len(a))
        nact.append(len(a))
    NVT = (max(nact) + P - 1) // P
    pl.NVT = NVT
    pl.NG2 = (NVT + GRP - 1) // GRP

    keep = lane[dst] >= 0
    row2, et2, norm2, owner2 = row[keep], et[keep], norm[keep], owner[keep]
    lane2 = lane[dst[keep]]
    vt = lane2 // P
    ct2 = (vt % GRP) * SKEY + et2 * P + (lane2 % P)
    sec2 = (row2 >= lolim).astype(np.int64)
    _schedule_layer.norm = norm2
    pl.L2 = _schedule_layer(row2, ct2, sec2, owner2, vt // GRP, pl.NG2,
                            cores, GKEY, hibase)

    # ---- h1T compaction pairs: (chunk tile, vtile) with any active node
    # active nodes of chunk t occupy a contiguous lane run -> few pairs.
    pairs = []   # list of (t, v); shared across cores (union)
    pair_set = set()
    for c in range(cores):
        a = pl.active[c]
        if not len(a):
            continue
        t_of = a // P
        v_of = np.arange(len(a)) // P
        for t, v in set(zip(t_of.tolist(), v_of.tolist())):
            if (t, v) not in pair_set:
                pair_set.add((t, v))
                pairs.append((t, v))
    pairs.sort()
    pl.pairs = pairs
    pl.NPAIR = len(pairs)
    # per-core selkeys [128, NPAIR]: lane within vtile v for node (t, p), else -1
    pl.selkeys = {}
    for c in range(cores):
        a = pl.active[c]
        sk = np.full((P, pl.NPAIR), -1.0, np.float32)
        if len(a):
            t_of = a // P
            p_of = a % P
            l_of = np.arange(len(a))
            pos = {(t, v): j for j, (t, v) in enumerate(pairs)}
            for t, p, l in zip(t_of.tolist(), p_of.tolist(), l_of.tolist()):
                j = pos.get((t, l // P))
                if j is not None:
                    sk[p, j] = l % P
        pl.selkeys[c] = np.ascontiguousarray(sk.astype(BF16))
    return pl


def wrap16(flat):
    """[L] int16 -> [128, L//16] wrapped layout for dma_gather idxs."""
    L = len(flat)
    assert L % 16 == 0
    a = np.asarray(flat, np.int16).reshape(-1, 16).T  # [16, L//16]
    return np.ascontiguousarray(np.tile(a, (8, 1)))


def blob_layout(pl):
    """Ordered (name, nelem, shape) segments of the single bf16 input blob.
    int16 segments are stored bit-cast as bf16. Offsets 128-elem aligned."""
    NSP, NT = pl.NSP, pl.NT
    TC = TEXT // P
    TOT = pl.L1.NSLOT + pl.L2.NSLOT
    segs = [
        ("textT", [NT, P, TC * P]),
        ("valT", [VAL, NSP]),
        ("fc1w", [VAL, FEAT]),
        ("fc2w", [P, TC * P]),
        ("rwv", [FEAT, FEAT]),
        ("rwt", [FEAT, FEAT]),
        ("beff", [1, FEAT]),
        ("ww1", [P, N_REL * FEAT]),
        ("wroot1", [P, FEAT]),
        ("b1", [1, FEAT]),
        ("ww2", [P, N_REL * FEAT]),
        ("wroot2", [P, FEAT]),
        ("b2", [1, FEAT]),
        ("fc3w", [FEAT, CLASSES]),
        ("fc3b", [1, CLASSES]),
        ("iota", [P, W]),
        ("iota128", [P, P]),
        ("ones1", [1, P]),
        ("keys", [P, max(TOT, 1)]),
        ("norms", [P, max(TOT, 1)]),
        ("selkeys", [P, max(pl.NPAIR, 1)]),
        ("idx1lo", [P, max(pl.L1.NLO, 1) * 8]),
        ("idx1hi", [P, max(pl.L1.NHI, 1) * 8]),
        ("idx2lo", [P, max(pl.L2.NLO, 1) * 8]),
        ("idx2hi", [P, max(pl.L2.NHI, 1) * 8]),
    ]
    out = {}
    off = 0
    for name, shape in segs:
        n = int(np.prod(shape))
        out[name] = (off, n, shape)
        off += ((n + 127) // 128) * 128
    return out, off

# ============================ bass builder =============================

def build_bass(pl, ablate=()):
    ab = set(ablate)
    NSP, NT, NROWS = pl.NSP, pl.NT, pl.NROWS
    NVT = pl.NVT
    TC = TEXT // P

    cores = getattr(pl, "cores", CORES)
    nc = bacc.Bacc("TRN2", target_bir_lowering=False, debug=False,
                   num_devices=cores, num_swdge_queues=4)
    qrr = {"n": 0}  # round-robin SWDGE queue picker
    dt = mybir.dt
    f32, bf, i16 = dt.float32, dt.bfloat16, dt.int16

    layout, blob_n = blob_layout(pl)
    p_blob = nc.declare_dram_parameter("blob", [1, blob_n], bf, isOutput=False)
    p_logT = nc.declare_dram_parameter("logitsT", [CLASSES, NVT * P], f32,
                                       isOutput=True)
    dbg = "dbg" in ab
    if dbg:
        p_dbg = nc.declare_dram_parameter(
            "dbg", [2 * NROWS + P, FEAT], bf, isOutput=True)

    def seg(name, dtype=bf):
        off, n, shape = layout[name]
        ap = p_blob[0:1, off:off + n]
        if dtype != bf:
            ap = ap.bitcast(dtype)
        r = int(np.prod(shape[:-1]))
        return ap.rearrange("o (r c) -> (o r) c", r=r)

    with tile.TileContext(nc) as tc:
        with tc.tile_pool(name="wt", bufs=1) as wt, \
             tc.tile_pool(name="sb", bufs=2) as sb, \
             tc.tile_pool(name="elo", bufs=10) as elo, \
             tc.tile_pool(name="ehi", bufs=10) as ehi, \
             tc.tile_pool(name="tts", bufs=3) as tts, \
             tc.tile_pool(name="dram", bufs=1, space="DRAM") as dram:

            def resident(name, dtype=bf):
                off, n, shape = layout[name]
                t = wt.tile(list(shape[-2:] if len(shape) == 2 else shape), dtype,
                            tag=name)
                nc.sync.dma_start(t[:], seg(name, dtype))
                return t

            fc1w = resident("fc1w")
            fc2w = resident("fc2w")
            rwv = resident("rwv")
            rwt = resident("rwt")
            beff = resident("beff")
            ww1 = resident("ww1")
            wroot1 = resident("wroot1")
            b1 = resident("b1")
            ww2 = resident("ww2")
            wroot2 = resident("wroot2")
            b2 = resident("b2")
            fc3w = resident("fc3w")
            fc3b = resident("fc3b")
            iota = resident("iota")
            iota128 = resident("iota128")
            ones1 = resident("ones1")
            valT = resident("valT")
            keys = resident("keys")
            norms = resident("norms")
            selkeys = resident("selkeys")
            idxsb = {1: [resident("idx1lo", i16), resident("idx1hi", i16)],
                     2: [resident("idx2lo", i16), resident("idx2hi", i16)]}

            # local-shard staging (partition-major) + transposed copy
            hstage = wt.tile([P, NT * P], bf, tag="hstage")
            hT0 = wt.tile([P, NT, P], bf, tag="hT0")
            h1Tc = wt.tile([P, max(NVT, 1) * P], bf, tag="h1Tc")

            # DRAM intermediates (row-major for the gather)
            h_shard = dram.tile([NSP, FEAT], bf)
            _as = "Shared" if (cores > 1 and "coll" not in ab) else "Local"
            h_full = dram.tile([NROWS, FEAT], bf, addr_space=_as)
            h1_shard = dram.tile([NSP, FEAT], bf)
            h1_full = dram.tile([NROWS, FEAT], bf, addr_space=_as)

            # ================= phase 1: feature MLP =================
            with tc.tile_pool(name="ps1", bufs=2, space="PSUM") as ps1:
                for t in range(NT):
                    tt = tts.tile([P, TC, P], bf, tag="tt")
                    toff = layout["textT"][0] + t * P * TC * P
                    nc.sync.dma_start(
                        tt[:], p_blob[0:1, toff:toff + P * TC * P]
                        .rearrange("o (p c n) -> (o p) c n", p=P, c=TC))
                    pvT = ps1.tile([P, P], f32, tag="pvT", space="PSUM")
                    nc.tensor.matmul(out=pvT[:], lhsT=fc1w[:],
                                     rhs=valT[:, t * P:(t + 1) * P],
                                     start=True, stop=True)
                    vT = sb.tile([P, P], bf, tag="vT")
                    nc.vector.tensor_copy(out=vT[:], in_=pvT[:])
                    ptT = ps1.tile([P, P], f32, tag="ptT", space="PSUM")
                    for c in range(TC):
                        nc.tensor.matmul(out=ptT[:],
                                         lhsT=fc2w[:, c * P:(c + 1) * P],
                                         rhs=tt[:, c, :],
                                         start=(c == 0), stop=(c == TC - 1))
                    tT = sb.tile([P, P], bf, tag="tT")
                    nc.vector.tensor_copy(out=tT[:], in_=ptT[:])
                    ph = ps1.tile([P, P], f32, tag="ph", space="PSUM")
                    nc.tensor.matmul(out=ph[:], lhsT=vT[:], rhs=rwv[:],
                                     start=True, stop=False)
                    nc.tensor.matmul(out=ph[:], lhsT=tT[:], rhs=rwt[:],
                                     start=False, stop=False)
                    nc.tensor.matmul(out=ph[:], lhsT=ones1[:], rhs=beff[:],
                                     start=False, stop=True)
                    lk = sb.tile([P, P], f32, tag="lk")
                    nc.vector.tensor_scalar(out=lk[:], in0=ph[:], scalar1=0.01,
                                            scalar2=None, op0=mybir.AluOpType.mult)
                    nc.vector.tensor_tensor(out=hstage[:, t * P:(t + 1) * P],
                                            in0=ph[:], in1=lk[:],
                                            op=mybir.AluOpType.max)

            def share_h(shard_dram, full_dram, xpose_to):
                """hstage -> row-major shard -> AllGather -> full; optional
                xbar transpose of the stage for the root term."""
                nc.sync.dma_start(
                    shard_dram[:].rearrange("(t p) f -> p t f", p=P),
                    hstage[:].rearrange("p (t f) -> p t f", t=NT))
                if cores > 1 and "coll" not in ab:
                    nc.gpsimd.collective_compute(
                        "AllGather", mybir.AluOpType.bypass,
                        replica_groups=[list(range(cores))],
                        ins=[shard_dram.opt()], outs=[full_dram.opt()])
                else:
                    nc.sync.dma_start(full_dram[0:NSP, :], shard_dram[:])
                if xpose_to is not None:
                    nc.scalar.dma_start(xpose_to[:], hstage[:], transpose=True)

            share_h(h_shard, h_full, hT0)

            # ================= RGCN layers =================
            def rgcn_layer(L, src_full, layer_idx, key_off, ww, wroot, bb):
                emitted = {0: -1, 1: -1}
                aemitted = {0: -1, 1: -1}
                ebufs = {0: {}, 1: {}}
                abufs = {0: {}, 1: {}}
                pools = {0: elo, 1: ehi}
                nstream = {0: L.NLO, 1: L.NHI}

                def emit_chunk(s, ci):
                    s0 = ci * CHMAX
                    ns = min(CHMAX, nstream[s] - s0)
                    et = pools[s].tile([P, CHMAX, FEAT], bf, tag=f"e{s}")
                    if "gather" in ab:
                        nc.vector.memset(et[:, 0:1, 0:2], 0.0)
                        ebufs[s][ci] = (et, s0, ns)
                        ebufs[s].pop(ci - 9, None)
                        return
                    if s == 0:
                        src_ap = src_full[0:pl.LOLIM, :]
                    else:
                        src_ap = src_full[pl.HIBASE:pl.NROWS, :]
                    qrr["n"] += 1
                    nc.gpsimd.dma_gather(
                        out_ap=et[:, 0:ns, :],
                        in_ap=src_ap,
                        idxs_ap=idxsb[layer_idx][s][:, s0 * 8:(s0 + ns) * 8],
                        num_idxs=ns * P,
                        num_idxs_reg=ns * P,
                        elem_size=FEAT,
                        queue_num=qrr["n"] % 4)
                    ebufs[s][ci] = (et, s0, ns)
                    ebufs[s].pop(ci - 9, None)

                def emit_abatch(s, ai):
                    s0 = ai * ABATCH
                    ns = min(ABATCH, nstream[s] - s0)
                    at = pools[s].tile([P, ABATCH, W], bf, tag=f"a{s}")
                    if "abuild" in ab:
                        nc.vector.memset(at[:, 0:1, 0:2], 0.0)
                        abufs[s][ai] = (at, s0, ns)
                        abufs[s].pop(ai - 3, None)
                        return
                    g0 = key_off + s0 + (0 if s == 0 else L.NLO)
                    kb = keys[:, g0:g0 + ns].unsqueeze(2).to_broadcast([P, ns, W])
                    nb = norms[:, g0:g0 + ns].unsqueeze(2).to_broadcast([P, ns, W])
                    ib = iota[:].unsqueeze(1).to_broadcast([P, ns, W])
                    nc.vector.tensor_tensor(out=at[:, 0:ns, :], in0=ib, in1=kb,
                                            op=mybir.AluOpType.is_equal)
                    nc.vector.tensor_tensor(out=at[:, 0:ns, :], in0=at[:, 0:ns, :],
                                            in1=nb, op=mybir.AluOpType.mult)
                    abufs[s][ai] = (at, s0, ns)
                    abufs[s].pop(ai - 3, None)

                n_groups = L.n_groups
                ntile = NT if layer_idx == 1 else NVT
                with tc.tile_pool(name=f"psl{layer_idx}", bufs=2,
                                  space="PSUM") as psl:
                    for g in range(n_groups):
                        tiles = [t for t in (GRP * g, GRP * g + 1) if t < ntile]
                        gw = (len(tiles) - 1) * SKEY + TKEY
                        pS = psl.tile([P, 8 * P], f32, tag="pS", space="PSUM")
                        if "memset" not in ab:
                            nc.vector.memset(pS[:, 0:gw], 0.0)
                        for s in (0, 1):
                            a, b = L.group_slot_range[s][g]
                            for j in range(a, b):
                                ci = j // CHMAX
                                ai = j // ABATCH
                                if ci > emitted[s]:
                                    emit_chunk(s, ci)
                                    emitted[s] = ci
                                if ai > aemitted[s]:
                                    emit_abatch(s, ai)
                                    aemitted[s] = ai
                                et, es0, _ = ebufs[s][ci]
                                at, as0, _ = abufs[s][ai]
                                bj = int(L.slot_base[s][j])
                                if "slotmm" in ab:
                                    continue
                                nc.tensor.matmul(
                                    out=pS[:, bj:bj + W],
                                    lhsT=et[:, j - es0, :], rhs=at[:, j - as0, :],
                                    start=False, stop=False,
                                    skip_group_check=True)
                        sS = sb.tile([P, GKEY], bf, tag="sS")
                        nc.scalar.activation(out=sS[:, 0:gw], in_=pS[:, 0:gw],
                                             func=mybir.ActivationFunctionType.Copy)
                        for ti, t in enumerate(tiles):
                            o = ti * SKEY
                            if layer_idx == 1:
                                pO = psl.tile([P, FEAT], f32, tag="pO",
                                              space="PSUM")
                                for r in range(N_REL):
                                    nc.tensor.matmul(
                                        out=pO[:],
                                        lhsT=sS[:, o + r * P:o + (r + 1) * P],
                                        rhs=ww[:, r * FEAT:(r + 1) * FEAT],
                                        start=(r == 0), stop=False)
                                nc.tensor.matmul(out=pO[:], lhsT=hT0[:, t, :],
                                                 rhs=wroot[:], start=False,
                                                 stop=False)
                                nc.tensor.matmul(out=pO[:], lhsT=ones1[:],
                                                 rhs=bb[:], start=False, stop=True)
                                nc.vector.tensor_copy(
                                    out=hstage[:, t * P:(t + 1) * P], in_=pO[:])
                            else:
                                pO = psl.tile([P, P], f32, tag="pO", space="PSUM")
                                for r in range(N_REL):
                                    nc.tensor.matmul(
                                        out=pO[:],
                                        lhsT=ww[:, r * FEAT:(r + 1) * FEAT],
                                        rhs=sS[:, o + r * P:o + (r + 1) * P],
                                        start=(r == 0), stop=False)
                                nc.tensor.matmul(out=pO[:], lhsT=wroot[:],
                                                 rhs=h1Tc[:, t * P:(t + 1) * P],
                                                 start=False, stop=False)
                                nc.tensor.matmul(out=pO[:], lhsT=bb[:],
                                                 rhs=ones1[:], start=False,
                                                 stop=True)
                                h2T = sb.tile([P, P], bf, tag="h2T")
                                nc.vector.tensor_copy(out=h2T[:], in_=pO[:])
                                pL = psl.tile([CLASSES, P], f32, tag="pL",
                                              space="PSUM")
                                nc.tensor.matmul(out=pL[:], lhsT=fc3w[:],
                                                 rhs=h2T[:], start=True,
                                                 stop=False)
                                nc.tensor.matmul(out=pL[:], lhsT=fc3b[:],
                                                 rhs=ones1[:], start=False,
                                                 stop=True)
                                lg = sb.tile([CLASSES, P], f32, tag="lg")
                                nc.vector.tensor_copy(out=lg[:], in_=pL[:])
                                nc.sync.dma_start(
                                    p_logT[:, t * P:(t + 1) * P], lg[:])

            rgcn_layer(pl.L1, h_full, 1, 0, ww1, wroot1, b1)
            share_h(h1_shard, h1_full, None)
            if dbg:
                nc.sync.dma_start(p_dbg[0:NROWS, :], h_full[:])
                nc.sync.dma_start(p_dbg[NROWS:2 * NROWS, :], h1_full[:])

            # h1T compaction: h1Tc[:, v*128+lane] = h1[active node, :]^T
            if pl.NPAIR:
                with tc.tile_pool(name="psc", bufs=1, space="PSUM") as psc:
                    sel = wt.tile([P, pl.NPAIR, P], bf, tag="sel")
                    skb = selkeys[:].unsqueeze(2).to_broadcast([P, pl.NPAIR, P])
                    i128 = iota128[:].unsqueeze(1).to_broadcast([P, pl.NPAIR, P])
                    nc.vector.tensor_tensor(out=sel[:], in0=i128, in1=skb,
                                            op=mybir.AluOpType.is_equal)
                    pC = psc.tile([P, max(NVT, 1) * P], f32, tag="pC",
                                  space="PSUM")
                    nc.vector.memset(pC[:], 0.0)
                    for j, (t, v) in enumerate(pl.pairs):
                        nc.tensor.matmul(
                            out=pC[:, v * P:(v + 1) * P],
                            lhsT=hstage[:, t * P:(t + 1) * P],
                            rhs=sel[:, j, :],
                            start=False, stop=False,
                            skip_group_check=True)
                    nc.vector.tensor_copy(out=h1Tc[:], in_=pC[:])
                    if dbg:
                        nc.sync.dma_start(p_dbg[2 * NROWS:2 * NROWS + P, :],
                                          h1Tc[:, 0:P])

            rgcn_layer(pl.L2, h1_full, 2, pl.L1.NSLOT, ww2, wroot2, b2)

    nc.compile()
    return nc


# ============================ host packing =============================

def pack_inputs(pl, inputs):
    """Build per-core in_maps from the full problem inputs."""
    NS, NSP, NT = pl.NS, pl.NSP, pl.NT
    TC = TEXT // P

    vf = np.asarray(inputs["value_feature"], np.float32)
    tf = np.asarray(inputs["text_feature"], np.float32)

    def shard_textT(c):
        x = np.zeros((NSP, TEXT), np.float32)
        x[:NS] = tf[c * NS:(c + 1) * NS]
        y = x.reshape(NT, P, TC, P).transpose(0, 3, 2, 1)
        return np.ascontiguousarray(y.reshape(NT, P, TC * P).astype(BF16))

    def shard_valT(c):
        x = np.zeros((NSP, VAL), np.float32)
        x[:NS] = vf[c * NS:(c + 1) * NS]
        return np.ascontiguousarray(x.T.astype(BF16))

    f32 = np.float32
    fc1w = np.asarray(inputs["fc1_w"], f32)
    fc2w = np.asarray(inputs["fc2_w"], f32)
    relw = np.asarray(inputs["relu_w"], f32)
    beff = (np.concatenate([np.asarray(inputs["fc1_b"], f32),
                            np.asarray(inputs["fc2_b"], f32)]) @ relw
            + np.asarray(inputs["relu_b"], f32))
    fc2w_t = np.ascontiguousarray(
        fc2w.reshape(TC, P, FEAT).transpose(1, 0, 2).reshape(P, TC * FEAT).astype(BF16))

    def stack_w(wrel):
        w = np.asarray(wrel, f32)
        return np.ascontiguousarray(
            w.transpose(1, 0, 2).reshape(P, N_REL * FEAT).astype(BF16))

    layout, blob_n = blob_layout(pl)
    shared = dict(
        fc1w=fc1w.astype(BF16), fc2w=fc2w_t,
        rwv=np.ascontiguousarray(relw[:FEAT].astype(BF16)),
        rwt=np.ascontiguousarray(relw[FEAT:].astype(BF16)),
        beff=beff[None].astype(BF16),
        ww1=stack_w(inputs["rgcn1_wrel"]),
        wroot1=np.asarray(inputs["rgcn1_wroot"], f32).astype(BF16),
        b1=np.asarray(inputs["rgcn1_b"], f32)[None].astype(BF16),
        ww2=stack_w(inputs["rgcn2_wrel"]),
        wroot2=np.asarray(inputs["rgcn2_wroot"], f32).astype(BF16),
        b2=np.asarray(inputs["rgcn2_b"], f32)[None].astype(BF16),
        fc3w=np.asarray(inputs["fc3_w"], f32).astype(BF16),
        fc3b=np.asarray(inputs["fc3_b"], f32)[None].astype(BF16),
        iota=np.tile(np.arange(W, dtype=f32), (P, 1)).astype(BF16),
        iota128=np.tile(np.arange(P, dtype=f32), (P, 1)).astype(BF16),
        ones1=np.ones((1, P), f32).astype(BF16),
    )

    def idxseg(arr):
        return (wrap16(arr.reshape(-1)) if arr.size
                else np.zeros((P, 8), np.int16)).view(BF16)

    in_maps = []
    for c in range(CORES):
        vals = dict(shared)
        vals["textT"] = shard_textT(c)
        vals["valT"] = shard_valT(c)
        vals["idx1lo"] = idxseg(pl.L1.idx_wrapped[c][0])
        vals["idx1hi"] = idxseg(pl.L1.idx_wrapped[c][1])
        vals["idx2lo"] = idxseg(pl.L2.idx_wrapped[c][0])
        vals["idx2hi"] = idxseg(pl.L2.idx_wrapped[c][1])
        kk = np.concatenate([pl.L1.keys[c], pl.L2.keys[c]], axis=1)
        nn = np.concatenate([pl.L1.norms[c], pl.L2.norms[c]], axis=1)
        vals["keys"] = kk if kk.size else np.zeros((P, 1), BF16)
        vals["norms"] = nn if nn.size else np.zeros((P, 1), BF16)
        vals["selkeys"] = (pl.selkeys[c] if pl.NPAIR
                           else np.zeros((P, 1), BF16))
        blob = np.zeros((1, blob_n), BF16)
        for name, (off, n, shape) in layout.items():
            a = vals[name]
            assert a.size == n, (name, a.shape, shape)
            blob[0, off:off + n] = a.reshape(-1)
        in_maps.append({"blob": blob})
    return in_maps


# ============================ entry point =============================

_cache = {}


def kernel(**inputs):
    ei = np.asarray(inputs["edge_index"], np.int64)
    et = np.asarray(inputs["edge_type"], np.int64)
    idx = np.asarray(inputs["idx"], np.int64)

    key = hash((ei.tobytes(), et.tobytes(), idx.tobytes()))
    if key not in _cache:
        pl = make_plan(ei, et, idx)
        nc = build_bass(pl)
        _cache[key] = (pl, nc)
    pl, nc = _cache[key]

    in_maps = pack_inputs(pl, inputs)
    res = run_bass_kernel_spmd(nc, in_maps, list(range(CORES)))

    NS = pl.NS
    logits = np.zeros((N_NODES, CLASSES), np.float32)
    for c in range(CORES):
        lt = res.results[c]["logitsT"]  # [2, NVT*128]
        a = pl.active[c]
        if len(a):
            logits[c * NS + a] = lt[:, :len(a)].T
    out = logits[idx]
    return out.astype(np.float32)


# revision 14
# speedup vs baseline: 6.1700x; 1.2023x over previous
"""BotRGCN (2-layer relational GCN) Trainium2 kernel, 8-way SPMD.

Strategy (per sharding hint): nodes sharded contiguously across 8 cores;
edges partitioned by destination core; relation weights replicated.

v3: the binding resource is SWDGE descriptor generation on the GpSimd Q7
(~7ns per gathered row, serialized), so the design minimizes gathered
lanes: (a) self-loops never enter the edge stream -- the root term is one
dense matmul per tile against an xbar-transposed copy of the local shard;
(b) layer 2 aggregates only at the 10000 output nodes (`idx`), remapped to
compact per-core "virtual tiles" whose logits the host scatters back;
(c) destination tiles are scheduled in pairs sharing one PSUM window so
slot fill improves.  Per 128-edge slot, one SWDGE dma_gather pulls source
rows from the AllGathered h table in DRAM (edge-major, no transpose
needed) and one PE matmul with a DVE-built one-hot A scatters them into
PSUM: S^T[f, key] += E^T @ A, key = rel*128 + dst_lane.

Self-contained: only imports the system concourse toolchain.
"""
import os
import sys

for _p in ("/opt/trn_rl_repo", "/root/.axon_site/_ro/trn_rl_repo"):
    if os.path.isdir(_p) and _p not in sys.path:
        sys.path.insert(0, _p)

import numpy as np
import ml_dtypes

from concourse import bass, bacc, tile, mybir
from concourse.bass_utils import run_bass_kernel_spmd

BF16 = ml_dtypes.bfloat16

# ---------------- problem constants (hardcoded per spec) ----------------
N_NODES = 50000
N_REL = 3
FEAT = 128
VAL = 16
TEXT = 768
CLASSES = 2
CORES = 8
P = 128           # partition / tile size
W = 96            # one-hot window width
CHMAX = 8         # slots per gather chunk (1024 idxs = SWDGE ring cap)
ABATCH = 16       # slots per A-matrix build batch
TKEY = N_REL * P  # per-tile key space: key = rel*128 + dst_lane
GRP = 2           # dst tiles per PSUM supergroup
SKEY = 512        # key stride between tiles in a group (PSUM bank aligned so
                  # no W-window ever crosses a 2KB bank boundary)
GKEY = (GRP - 1) * SKEY + TKEY


# ============================ host planner =============================

def _build_schedule(cts, cmax):
    """Joint (cross-core) slot schedule for one (group, section).

    cts: list of 8 sorted int arrays (edge keys in [0, cmax)).
    Returns (bases, ranges) where bases[j] is the shared window base of
    slot j and ranges[c][j] = (start, end) into core c's sorted arrays.
    """
    n = len(cts)
    ptrs = [0] * n
    lens = [len(a) for a in cts]
    bases = []
    ranges = [[] for _ in range(n)]
    while any(ptrs[c] < lens[c] for c in range(n)):
        b = min(cts[c][ptrs[c]] for c in range(n) if ptrs[c] < lens[c])
        b = min(int(b), cmax - W)
        bases.append(b)
        for c in range(n):
            s = ptrs[c]
            hi = int(np.searchsorted(cts[c], b + W, side="left"))
            e = min(s + P, hi)
            e = max(e, s)
            ranges[c].append((s, e))
            ptrs[c] = e
    return bases, ranges


class Layer:
    pass


class Plan:
    pass


def _schedule_layer(row, ct, sec, owner, group, n_groups, cores, key_span,
                    hibase):
    """Joint slot schedule over (group, sec); returns a Layer."""
    L = Layer()
    L.n_groups = n_groups
    order = np.lexsort((ct, sec, group, owner))
    row, ct, sec, owner, group = (a[order] for a in (row, ct, sec, owner, group))
    norm = _schedule_layer.norm[order]

    key = (owner * n_groups + group) * 2 + sec
    bounds = np.searchsorted(key, np.arange(cores * n_groups * 2 + 1))

    def grp(c, g, s):
        k = (c * n_groups + g) * 2 + s
        return bounds[k], bounds[k + 1]

    slot_base = {0: [], 1: []}
    idx16 = {0: [[] for _ in range(cores)], 1: [[] for _ in range(cores)]}
    keyd = {0: [[] for _ in range(cores)], 1: [[] for _ in range(cores)]}
    nrmd = {0: [[] for _ in range(cores)], 1: [[] for _ in range(cores)]}
    group_slot_range = {0: np.zeros((n_groups, 2), np.int64),
                        1: np.zeros((n_groups, 2), np.int64)}

    for g in range(n_groups):
        for s in (0, 1):
            cts, rows_, nrms_ = [], [], []
            for c in range(cores):
                a, b = grp(c, g, s)
                cts.append(ct[a:b])
                rows_.append(row[a:b])
                nrms_.append(norm[a:b])
            start = len(slot_base[s])
            bases, ranges = _build_schedule(cts, key_span)
            for bj in bases:
                slot_base[s].append(bj)
            for c in range(cores):
                for j, (a, b) in enumerate(ranges[c]):
                    n = b - a
                    ii = np.zeros(P, np.int16)
                    kk = np.full(P, -1.0, np.float32)
                    nn = np.zeros(P, np.float32)
                    r = rows_[c][a:b]
                    if s == 1:
                        r = r - hibase
                    ii[:n] = r.astype(np.int16)
                    kk[:n] = (cts[c][a:b] - bases[j]).astype(np.float32)
                    nn[:n] = nrms_[c][a:b]
                    idx16[s][c].append(ii)
                    keyd[s][c].append(kk)
                    nrmd[s][c].append(nn)
            group_slot_range[s][g] = (start, len(slot_base[s]))

    L.NLO = len(slot_base[0])
    L.NHI = len(slot_base[1])
    L.NSLOT = L.NLO + L.NHI
    L.slot_base = {s: np.array(slot_base[s], np.int64) for s in (0, 1)}
    L.group_slot_range = group_slot_range
    L.idx_wrapped = {c: [(np.stack(idx16[s][c]) if idx16[s][c]
                          else np.zeros((0, P), np.int16)) for s in (0, 1)]
                     for c in range(cores)}
    L.keys = {}
    L.norms = {}
    for c in range(cores):
        kk = np.concatenate(
            [np.stack(keyd[s][c]) if keyd[s][c] else np.zeros((0, P), np.float32)
             for s in (0, 1)])
        nn = np.concatenate(
            [np.stack(nrmd[s][c]) if nrmd[s][c] else np.zeros((0, P), np.float32)
             for s in (0, 1)])
        L.keys[c] = np.ascontiguousarray(kk.T.astype(BF16))   # [128, NSLOT]
        L.norms[c] = np.ascontiguousarray(nn.T.astype(BF16))  # [128, NSLOT]
    return L


def make_plan(edge_index, edge_type, out_idx, n_nodes=N_NODES, cores=CORES,
              lolim=None):
    pl = Plan()
    pl.cores = cores
    NS = n_nodes // cores
    assert NS * cores == n_nodes
    NSP = ((NS + P - 1) // P) * P
    NT = NSP // P
    NROWS = cores * NSP
    if lolim is None:
        lolim = min(NROWS, 32768)
    hibase = max(0, NROWS - 32768)
    assert hibase <= lolim and hibase % P == 0
    pl.NS, pl.NSP, pl.NT, pl.NROWS = NS, NSP, NT, NROWS
    pl.LOLIM, pl.HIBASE = lolim, hibase
    pl.NG1 = (NT + GRP - 1) // GRP

    src = np.asarray(edge_index[0], np.int64)
    dst = np.asarray(edge_index[1], np.int64)
    et = np.asarray(edge_type, np.int64)

    deg = np.zeros((N_REL, n_nodes), np.int64)
    np.add.at(deg, (et, dst), 1)
    norm = 1.0 / np.maximum(deg[et, dst], 1).astype(np.float32)

    row = (src // NS) * NSP + (src % NS)
    owner = dst // NS
    loc = dst % NS
    tile_id = loc // P
    sec = (row >= lolim).astype(np.int64)

    # ---- layer 2 active nodes first (layer 1 only needs h1 at sources of
    # kept layer-2 edges plus the active nodes themselves)
    oi = np.unique(np.asarray(out_idx, np.int64))
    pl.active = {}   # core -> sorted local node ids
    lane = np.full(n_nodes, -1, np.int64)   # node -> global lane on its core
    nact = []
    for c in range(cores):
        a = oi[(oi >= c * NS) & (oi < (c + 1) * NS)] - c * NS
        pl.active[c] = a
        lane[c * NS + a] = np.arange(len(a))
        nact.append(len(a))
    NVT = (max(nact) + P - 1) // P
    pl.NVT = NVT
    pl.NG2 = (NVT + GRP - 1) // GRP

    keep = lane[dst] >= 0
    row2, et2, norm2, owner2 = row[keep], et[keep], norm[keep], owner[keep]
    lane2 = lane[dst[keep]]
    vt = lane2 // P
    ct2 = (vt % GRP) * SKEY + et2 * P + (lane2 % P)
    sec2 = (row2 >= lolim).astype(np.int64)
    _schedule_layer.norm = norm2
    pl.L2 = _schedule_layer(row2, ct2, sec2, owner2, vt // GRP, pl.NG2,
                            cores, GKEY, hibase)

    # ---- layer 1: edges whose dst row feeds layer 2, groups = tile pairs
    need = np.zeros(n_nodes, bool)
    need[src[keep]] = True   # sources of kept layer-2 edges
    need[oi] = True          # root-term reads at active nodes
    keep1 = need[dst]
    row1, et1, norm1, owner1 = row[keep1], et[keep1], norm[keep1], owner[keep1]
    loc1, tile1 = loc[keep1], tile_id[keep1]
    sec1 = sec[keep1]
    ct1 = (tile1 % GRP) * SKEY + et1 * P + (loc1 % P)
    _schedule_layer.norm = norm1
    pl.L1 = _schedule_layer(row1, ct1, sec1, owner1, tile1 // GRP, pl.NG1,
                            cores, GKEY, hibase)

    # ---- h1T compaction pairs: (chunk tile, vtile) with any active node
    # active nodes of chunk t occupy a contiguous lane run -> few pairs.
    pairs = []   # list of (t, v); shared across cores (union)
    pair_set = set()
    for c in range(cores):
        a = pl.active[c]
        if not len(a):
            continue
        t_of = a // P
        v_of = np.arange(len(a)) // P
        for t, v in set(zip(t_of.tolist(), v_of.tolist())):
            if (t, v) not in pair_set:
                pair_set.add((t, v))
                pairs.append((t, v))
    pairs.sort()
    pl.pairs = pairs
    pl.NPAIR = len(pairs)
    # per-core selkeys [128, NPAIR]: lane within vtile v for node (t, p), else -1
    pl.selkeys = {}
    for c in range(cores):
        a = pl.active[c]
        sk = np.full((P, pl.NPAIR), -1.0, np.float32)
        if len(a):
            t_of = a // P
            p_of = a % P
            l_of = np.arange(len(a))
            pos = {(t, v): j for j, (t, v) in enumerate(pairs)}
            for t, p, l in zip(t_of.tolist(), p_of.tolist(), l_of.tolist()):
                j = pos.get((t, l // P))
                if j is not None:
                    sk[p, j] = l % P
        pl.selkeys[c] = np.ascontiguousarray(sk.astype(BF16))
    return pl


def wrap16(flat):
    """[L] int16 -> [128, L//16] wrapped layout for dma_gather idxs."""
    L = len(flat)
    assert L % 16 == 0
    a = np.asarray(flat, np.int16).reshape(-1, 16).T  # [16, L//16]
    return np.ascontiguousarray(np.tile(a, (8, 1)))


def blob_layout(pl):
    """Ordered (name, nelem, shape) segments of the single bf16 input blob.
    int16 segments are stored bit-cast as bf16. Offsets 128-elem aligned."""
    NSP, NT = pl.NSP, pl.NT
    TC = TEXT // P
    TOT = pl.L1.NSLOT + pl.L2.NSLOT
    segs = [
        ("textT", [NT, P, TC * P]),
        ("valT", [VAL, NSP]),
        ("fc1w", [VAL, FEAT]),
        ("fc2w", [P, TC * P]),
        ("rwv", [FEAT, FEAT]),
        ("rwt", [FEAT, FEAT]),
        ("beff", [1, FEAT]),
        ("ww1", [P, N_REL * FEAT]),
        ("wroot1", [P, FEAT]),
        ("b1", [1, FEAT]),
        ("ww2", [P, N_REL * FEAT]),
        ("wroot2", [P, FEAT]),
        ("b2", [1, FEAT]),
        ("fc3w", [FEAT, CLASSES]),
        ("fc3b", [1, CLASSES]),
        ("iota", [P, W]),
        ("iota128", [P, P]),
        ("ones1", [1, P]),
        ("keys", [P, max(TOT, 1)]),
        ("norms", [P, max(TOT, 1)]),
        ("selkeys", [P, max(pl.NPAIR, 1)]),
        ("idx1lo", [P, max(pl.L1.NLO, 1) * 8]),
        ("idx1hi", [P, max(pl.L1.NHI, 1) * 8]),
        ("idx2lo", [P, max(pl.L2.NLO, 1) * 8]),
        ("idx2hi", [P, max(pl.L2.NHI, 1) * 8]),
    ]
    out = {}
    off = 0
    for name, shape in segs:
        n = int(np.prod(shape))
        out[name] = (off, n, shape)
        off += ((n + 127) // 128) * 128
    return out, off

# ============================ bass builder =============================

def build_bass(pl, ablate=()):
    ab = set(ablate)
    NSP, NT, NROWS = pl.NSP, pl.NT, pl.NROWS
    NVT = pl.NVT
    TC = TEXT // P

    cores = getattr(pl, "cores", CORES)
    nc = bacc.Bacc("TRN2", target_bir_lowering=False, debug=False,
                   num_devices=cores, num_swdge_queues=4)
    qrr = {"n": 0}  # round-robin SWDGE queue picker
    dt = mybir.dt
    f32, bf, i16 = dt.float32, dt.bfloat16, dt.int16

    layout, blob_n = blob_layout(pl)
    p_blob = nc.declare_dram_parameter("blob", [1, blob_n], bf, isOutput=False)
    p_logT = nc.declare_dram_parameter("logitsT", [CLASSES, NVT * P], f32,
                                       isOutput=True)
    dbg = "dbg" in ab
    if dbg:
        p_dbg = nc.declare_dram_parameter(
            "dbg", [2 * NROWS + P, FEAT], bf, isOutput=True)

    def seg(name, dtype=bf):
        off, n, shape = layout[name]
        ap = p_blob[0:1, off:off + n]
        if dtype != bf:
            ap = ap.bitcast(dtype)
        r = int(np.prod(shape[:-1]))
        return ap.rearrange("o (r c) -> (o r) c", r=r)

    with tile.TileContext(nc) as tc:
        with tc.tile_pool(name="wt", bufs=1) as wt, \
             tc.tile_pool(name="sb", bufs=2) as sb, \
             tc.tile_pool(name="elo", bufs=8) as elo, \
             tc.tile_pool(name="ehi", bufs=8) as ehi, \
             tc.tile_pool(name="tts", bufs=3) as tts, \
             tc.tile_pool(name="dram", bufs=1, space="DRAM") as dram:

            def resident(name, dtype=bf):
                off, n, shape = layout[name]
                t = wt.tile(list(shape[-2:] if len(shape) == 2 else shape), dtype,
                            tag=name)
                nc.sync.dma_start(t[:], seg(name, dtype))
                return t

            fc1w = resident("fc1w")
            fc2w = resident("fc2w")
            rwv = resident("rwv")
            rwt = resident("rwt")
            beff = resident("beff")
            ww1 = resident("ww1")
            wroot1 = resident("wroot1")
            b1 = resident("b1")
            ww2 = resident("ww2")
            wroot2 = resident("wroot2")
            b2 = resident("b2")
            fc3w = resident("fc3w")
            fc3b = resident("fc3b")
            iota = resident("iota")
            iota128 = resident("iota128")
            ones1 = resident("ones1")
            valT = resident("valT")
            keys = resident("keys")
            norms = resident("norms")
            selkeys = resident("selkeys")
            idxsb = {1: [resident("idx1lo", i16), resident("idx1hi", i16)],
                     2: [resident("idx2lo", i16), resident("idx2hi", i16)]}

            # local-shard staging (partition-major) + transposed copy
            hstage = wt.tile([P, NT * P], bf, tag="hstage")
            hT0 = wt.tile([P, NT, P], bf, tag="hT0")
            h1Tc = wt.tile([P, max(NVT, 1) * P], bf, tag="h1Tc")

            # DRAM intermediates (row-major for the gather)
            h_shard = dram.tile([NSP, FEAT], bf)
            _as = "Shared" if (cores > 1 and "coll" not in ab) else "Local"
            h_full = dram.tile([NROWS, FEAT], bf, addr_space=_as)
            h1_shard = dram.tile([NSP, FEAT], bf)
            h1_full = dram.tile([NROWS, FEAT], bf, addr_space=_as)

            # ================= phase 1: feature MLP =================
            with tc.tile_pool(name="ps1", bufs=2, space="PSUM") as ps1:
                for t in range(NT):
                    tt = tts.tile([P, TC, P], bf, tag="tt")
                    toff = layout["textT"][0] + t * P * TC * P
                    nc.sync.dma_start(
                        tt[:], p_blob[0:1, toff:toff + P * TC * P]
                        .rearrange("o (p c n) -> (o p) c n", p=P, c=TC))
                    pvT = ps1.tile([P, P], f32, tag="pvT", space="PSUM")
                    nc.tensor.matmul(out=pvT[:], lhsT=fc1w[:],
                                     rhs=valT[:, t * P:(t + 1) * P],
                                     start=True, stop=True)
                    vT = sb.tile([P, P], bf, tag="vT")
                    nc.scalar.activation(out=vT[:], in_=pvT[:],
                                         func=mybir.ActivationFunctionType.Copy)
                    ptT = ps1.tile([P, P], f32, tag="ptT", space="PSUM")
                    for c in range(TC):
                        nc.tensor.matmul(out=ptT[:],
                                         lhsT=fc2w[:, c * P:(c + 1) * P],
                                         rhs=tt[:, c, :],
                                         start=(c == 0), stop=(c == TC - 1))
                    tT = sb.tile([P, P], bf, tag="tT")
                    nc.scalar.activation(out=tT[:], in_=ptT[:],
                                         func=mybir.ActivationFunctionType.Copy)
                    ph = ps1.tile([P, P], f32, tag="ph", space="PSUM")
                    nc.tensor.matmul(out=ph[:], lhsT=vT[:], rhs=rwv[:],
                                     start=True, stop=False)
                    nc.tensor.matmul(out=ph[:], lhsT=tT[:], rhs=rwt[:],
                                     start=False, stop=False)
                    nc.tensor.matmul(out=ph[:], lhsT=ones1[:], rhs=beff[:],
                                     start=False, stop=True)
                    lk = sb.tile([P, P], f32, tag="lk")
                    nc.scalar.activation(out=lk[:], in_=ph[:],
                                         func=mybir.ActivationFunctionType.Copy,
                                         scale=0.01)
                    nc.vector.tensor_tensor(out=hstage[:, t * P:(t + 1) * P],
                                            in0=ph[:], in1=lk[:],
                                            op=mybir.AluOpType.max)

            def share_h(shard_dram, full_dram, xpose_to):
                """hstage -> row-major shard -> AllGather -> full; optional
                xbar transpose of the stage for the root term."""
                nc.sync.dma_start(
                    shard_dram[:].rearrange("(t p) f -> p t f", p=P),
                    hstage[:].rearrange("p (t f) -> p t f", t=NT))
                if cores > 1 and "coll" not in ab:
                    nc.gpsimd.collective_compute(
                        "AllGather", mybir.AluOpType.bypass,
                        replica_groups=[list(range(cores))],
                        ins=[shard_dram.opt()], outs=[full_dram.opt()])
                else:
                    nc.sync.dma_start(full_dram[0:NSP, :], shard_dram[:])
                if xpose_to is not None:
                    nc.scalar.dma_start(xpose_to[:], hstage[:], transpose=True)

            share_h(h_shard, h_full, hT0)

            # ================= RGCN layers =================
            def rgcn_layer(L, src_full, layer_idx, key_off, ww, wroot, bb):
                emitted = {0: -1, 1: -1}
                aemitted = {0: -1, 1: -1}
                ebufs = {0: {}, 1: {}}
                abufs = {0: {}, 1: {}}
                pools = {0: elo, 1: ehi}
                nstream = {0: L.NLO, 1: L.NHI}

                def emit_chunk(s, ci):
                    s0 = ci * CHMAX
                    ns = min(CHMAX, nstream[s] - s0)
                    et = pools[s].tile([P, CHMAX, FEAT], bf, tag=f"e{s}")
                    if "gather" in ab:
                        nc.vector.memset(et[:, 0:1, 0:2], 0.0)
                        ebufs[s][ci] = (et, s0, ns)
                        ebufs[s].pop(ci - 8, None)
                        return
                    if s == 0:
                        src_ap = src_full[0:pl.LOLIM, :]
                    else:
                        src_ap = src_full[pl.HIBASE:pl.NROWS, :]
                    qrr["n"] += 1
                    nc.gpsimd.dma_gather(
                        out_ap=et[:, 0:ns, :],
                        in_ap=src_ap,
                        idxs_ap=idxsb[layer_idx][s][:, s0 * 8:(s0 + ns) * 8],
                        num_idxs=ns * P,
                        num_idxs_reg=ns * P,
                        elem_size=FEAT,
                        queue_num=qrr["n"] % 4)
                    ebufs[s][ci] = (et, s0, ns)
                    ebufs[s].pop(ci - 8, None)

                def emit_abatch(s, ai):
                    s0 = ai * ABATCH
                    ns = min(ABATCH, nstream[s] - s0)
                    at = pools[s].tile([P, ABATCH, W], bf, tag=f"a{s}")
                    if "abuild" in ab:
                        nc.vector.memset(at[:, 0:1, 0:2], 0.0)
                        abufs[s][ai] = (at, s0, ns)
                        abufs[s].pop(ai - 3, None)
                        return
                    g0 = key_off + s0 + (0 if s == 0 else L.NLO)
                    kb = keys[:, g0:g0 + ns].unsqueeze(2).to_broadcast([P, ns, W])
                    nb = norms[:, g0:g0 + ns].unsqueeze(2).to_broadcast([P, ns, W])
                    ib = iota[:].unsqueeze(1).to_broadcast([P, ns, W])
                    nc.vector.tensor_tensor(out=at[:, 0:ns, :], in0=ib, in1=kb,
                                            op=mybir.AluOpType.is_equal)
                    nc.vector.tensor_tensor(out=at[:, 0:ns, :], in0=at[:, 0:ns, :],
                                            in1=nb, op=mybir.AluOpType.mult)
                    abufs[s][ai] = (at, s0, ns)
                    abufs[s].pop(ai - 3, None)

                n_groups = L.n_groups
                ntile = NT if layer_idx == 1 else NVT
                with tc.tile_pool(name=f"psl{layer_idx}", bufs=2,
                                  space="PSUM") as psl:
                    for g in range(n_groups):
                        tiles = [t for t in (GRP * g, GRP * g + 1) if t < ntile]
                        gw = (len(tiles) - 1) * SKEY + TKEY
                        pS = psl.tile([P, 8 * P], f32, tag="pS", space="PSUM")
                        if "memset" not in ab:
                            nc.vector.memset(pS[:, 0:gw], 0.0)
                        for s in (0, 1):
                            a, b = L.group_slot_range[s][g]
                            for j in range(a, b):
                                ci = j // CHMAX
                                ai = j // ABATCH
                                if ci > emitted[s]:
                                    emit_chunk(s, ci)
                                    emitted[s] = ci
                                if ai > aemitted[s]:
                                    emit_abatch(s, ai)
                                    aemitted[s] = ai
                                et, es0, _ = ebufs[s][ci]
                                at, as0, _ = abufs[s][ai]
                                bj = int(L.slot_base[s][j])
                                if "slotmm" in ab:
                                    continue
                                nc.tensor.matmul(
                                    out=pS[:, bj:bj + W],
                                    lhsT=et[:, j - es0, :], rhs=at[:, j - as0, :],
                                    start=False, stop=False,
                                    skip_group_check=True)
                        sS = sb.tile([P, GKEY], bf, tag="sS")
                        nc.scalar.activation(out=sS[:, 0:gw], in_=pS[:, 0:gw],
                                             func=mybir.ActivationFunctionType.Copy)
                        for ti, t in enumerate(tiles):
                            o = ti * SKEY
                            if layer_idx == 1:
                                pO = psl.tile([P, FEAT], f32, tag="pO",
                                              space="PSUM")
                                for r in range(N_REL):
                                    nc.tensor.matmul(
                                        out=pO[:],
                                        lhsT=sS[:, o + r * P:o + (r + 1) * P],
                                        rhs=ww[:, r * FEAT:(r + 1) * FEAT],
                                        start=(r == 0), stop=False)
                                nc.tensor.matmul(out=pO[:], lhsT=hT0[:, t, :],
                                                 rhs=wroot[:], start=False,
                                                 stop=False)
                                nc.tensor.matmul(out=pO[:], lhsT=ones1[:],
                                                 rhs=bb[:], start=False, stop=True)
                                nc.vector.tensor_copy(
                                    out=hstage[:, t * P:(t + 1) * P], in_=pO[:])
                            else:
                                pO = psl.tile([P, P], f32, tag="pO", space="PSUM")
                                for r in range(N_REL):
                                    nc.tensor.matmul(
                                        out=pO[:],
                                        lhsT=ww[:, r * FEAT:(r + 1) * FEAT],
                                        rhs=sS[:, o + r * P:o + (r + 1) * P],
                                        start=(r == 0), stop=False)
                                nc.tensor.matmul(out=pO[:], lhsT=wroot[:],
                                                 rhs=h1Tc[:, t * P:(t + 1) * P],
                                                 start=False, stop=False)
                                nc.tensor.matmul(out=pO[:], lhsT=bb[:],
                                                 rhs=ones1[:], start=False,
                                                 stop=True)
                                h2T = sb.tile([P, P], bf, tag="h2T")
                                nc.vector.tensor_copy(out=h2T[:], in_=pO[:])
                                pL = psl.tile([CLASSES, P], f32, tag="pL",
                                              space="PSUM")
                                nc.tensor.matmul(out=pL[:], lhsT=fc3w[:],
                                                 rhs=h2T[:], start=True,
                                                 stop=False)
                                nc.tensor.matmul(out=pL[:], lhsT=fc3b[:],
                                                 rhs=ones1[:], start=False,
                                                 stop=True)
                                lg = sb.tile([CLASSES, P], f32, tag="lg")
                                nc.vector.tensor_copy(out=lg[:], in_=pL[:])
                                nc.sync.dma_start(
                                    p_logT[:, t * P:(t + 1) * P], lg[:])

            rgcn_layer(pl.L1, h_full, 1, 0, ww1, wroot1, b1)
            share_h(h1_shard, h1_full, None)
            if dbg:
                nc.sync.dma_start(p_dbg[0:NROWS, :], h_full[:])
                nc.sync.dma_start(p_dbg[NROWS:2 * NROWS, :], h1_full[:])

            # h1T compaction: h1Tc[:, v*128+lane] = h1[active node, :]^T
            if pl.NPAIR:
                with tc.tile_pool(name="psc", bufs=1, space="PSUM") as psc:
                    sel = wt.tile([P, pl.NPAIR, P], bf, tag="sel")
                    skb = selkeys[:].unsqueeze(2).to_broadcast([P, pl.NPAIR, P])
                    i128 = iota128[:].unsqueeze(1).to_broadcast([P, pl.NPAIR, P])
                    nc.vector.tensor_tensor(out=sel[:], in0=i128, in1=skb,
                                            op=mybir.AluOpType.is_equal)
                    pC = psc.tile([P, max(NVT, 1) * P], f32, tag="pC",
                                  space="PSUM")
                    nc.vector.memset(pC[:], 0.0)
                    for j, (t, v) in enumerate(pl.pairs):
                        nc.tensor.matmul(
                            out=pC[:, v * P:(v + 1) * P],
                            lhsT=hstage[:, t * P:(t + 1) * P],
                            rhs=sel[:, j, :],
                            start=False, stop=False,
                            skip_group_check=True)
                    nc.vector.tensor_copy(out=h1Tc[:], in_=pC[:])
                    if dbg:
                        nc.sync.dma_start(p_dbg[2 * NROWS:2 * NROWS + P, :],
                                          h1Tc[:, 0:P])

            rgcn_layer(pl.L2, h1_full, 2, pl.L1.NSLOT, ww2, wroot2, b2)

    nc.compile()
    return nc


# ============================ host packing =============================

def pack_inputs(pl, inputs):
    """Build per-core in_maps from the full problem inputs."""
    NS, NSP, NT = pl.NS, pl.NSP, pl.NT
    TC = TEXT // P

    vf = np.asarray(inputs["value_feature"], np.float32)
    tf = np.asarray(inputs["text_feature"], np.float32)

    def shard_textT(c):
        x = np.zeros((NSP, TEXT), np.float32)
        x[:NS] = tf[c * NS:(c + 1) * NS]
        y = x.reshape(NT, P, TC, P).transpose(0, 3, 2, 1)
        return np.ascontiguousarray(y.reshape(NT, P, TC * P).astype(BF16))

    def shard_valT(c):
        x = np.zeros((NSP, VAL), np.float32)
        x[:NS] = vf[c * NS:(c + 1) * NS]
        return np.ascontiguousarray(x.T.astype(BF16))

    f32 = np.float32
    fc1w = np.asarray(inputs["fc1_w"], f32)
    fc2w = np.asarray(inputs["fc2_w"], f32)
    relw = np.asarray(inputs["relu_w"], f32)
    beff = (np.concatenate([np.asarray(inputs["fc1_b"], f32),
                            np.asarray(inputs["fc2_b"], f32)]) @ relw
            + np.asarray(inputs["relu_b"], f32))
    fc2w_t = np.ascontiguousarray(
        fc2w.reshape(TC, P, FEAT).transpose(1, 0, 2).reshape(P, TC * FEAT).astype(BF16))

    def stack_w(wrel):
        w = np.asarray(wrel, f32)
        return np.ascontiguousarray(
            w.transpose(1, 0, 2).reshape(P, N_REL * FEAT).astype(BF16))

    layout, blob_n = blob_layout(pl)
    shared = dict(
        fc1w=fc1w.astype(BF16), fc2w=fc2w_t,
        rwv=np.ascontiguousarray(relw[:FEAT].astype(BF16)),
        rwt=np.ascontiguousarray(relw[FEAT:].astype(BF16)),
        beff=beff[None].astype(BF16),
        ww1=stack_w(inputs["rgcn1_wrel"]),
        wroot1=np.asarray(inputs["rgcn1_wroot"], f32).astype(BF16),
        b1=np.asarray(inputs["rgcn1_b"], f32)[None].astype(BF16),
        ww2=stack_w(inputs["rgcn2_wrel"]),
        wroot2=np.asarray(inputs["rgcn2_wroot"], f32).astype(BF16),
        b2=np.asarray(inputs["rgcn2_b"], f32)[None].astype(BF16),
        fc3w=np.asarray(inputs["fc3_w"], f32).astype(BF16),
        fc3b=np.asarray(inputs["fc3_b"], f32)[None].astype(BF16),
        iota=np.tile(np.arange(W, dtype=f32), (P, 1)).astype(BF16),
        iota128=np.tile(np.arange(P, dtype=f32), (P, 1)).astype(BF16),
        ones1=np.ones((1, P), f32).astype(BF16),
    )

    def idxseg(arr):
        return (wrap16(arr.reshape(-1)) if arr.size
                else np.zeros((P, 8), np.int16)).view(BF16)

    in_maps = []
    for c in range(CORES):
        vals = dict(shared)
        vals["textT"] = shard_textT(c)
        vals["valT"] = shard_valT(c)
        vals["idx1lo"] = idxseg(pl.L1.idx_wrapped[c][0])
        vals["idx1hi"] = idxseg(pl.L1.idx_wrapped[c][1])
        vals["idx2lo"] = idxseg(pl.L2.idx_wrapped[c][0])
        vals["idx2hi"] = idxseg(pl.L2.idx_wrapped[c][1])
        kk = np.concatenate([pl.L1.keys[c], pl.L2.keys[c]], axis=1)
        nn = np.concatenate([pl.L1.norms[c], pl.L2.norms[c]], axis=1)
        vals["keys"] = kk if kk.size else np.zeros((P, 1), BF16)
        vals["norms"] = nn if nn.size else np.zeros((P, 1), BF16)
        vals["selkeys"] = (pl.selkeys[c] if pl.NPAIR
                           else np.zeros((P, 1), BF16))
        blob = np.zeros((1, blob_n), BF16)
        for name, (off, n, shape) in layout.items():
            a = vals[name]
            assert a.size == n, (name, a.shape, shape)
            blob[0, off:off + n] = a.reshape(-1)
        in_maps.append({"blob": blob})
    return in_maps


# ============================ entry point =============================

_cache = {}


def kernel(**inputs):
    ei = np.asarray(inputs["edge_index"], np.int64)
    et = np.asarray(inputs["edge_type"], np.int64)
    idx = np.asarray(inputs["idx"], np.int64)

    key = hash((ei.tobytes(), et.tobytes(), idx.tobytes()))
    if key not in _cache:
        pl = make_plan(ei, et, idx)
        nc = build_bass(pl)
        _cache[key] = (pl, nc)
    pl, nc = _cache[key]

    in_maps = pack_inputs(pl, inputs)
    res = run_bass_kernel_spmd(nc, in_maps, list(range(CORES)))

    NS = pl.NS
    logits = np.zeros((N_NODES, CLASSES), np.float32)
    for c in range(CORES):
        lt = res.results[c]["logitsT"]  # [2, NVT*128]
        a = pl.active[c]
        if len(a):
            logits[c * NS + a] = lt[:, :len(a)].T
    out = logits[idx]
    return out.astype(np.float32)
